# revision 14
# baseline (speedup 1.0000x reference)
"""AKT dense-transformer forward on 8 Trainium2 NeuronCores.

Data-parallel over batch (2 sequences per core, no collectives).
Host precomputes the Rasch embeddings (pure input prep, ~0.1% of FLOPs),
weight layout transforms and -softplus(gamma); the device kernel runs the
6 attention blocks (monotonic-attention distance decay), 4 FFNs and the
prediction head.

Device layout: activations are feature-major ([D on partitions, tokens on
free]) so every linear is a natural PE matmul; attention scores stay
q-major for the softmax/decay chain; P is transposed per 128x128 block via
DRAM-round-trip DMA transpose for the PV matmul.  Matmuls run in float32r
(full-speed, tf32-like), the score chain in bf16.
"""

import numpy as np
import ml_dtypes

import concourse.bacc as bacc
import concourse.mybir as mybir
from concourse.tile import TileContext
from concourse.bass_utils import run_bass_kernel_spmd

# ---------------------------------------------------------------- constants
B, L = 16, 512
D, H = 512, 8
DH = D // H
NB = 2
FF = 2048
FC = 512
C = 1024
Q = 20000
NCORES = 8
BLOC = B // NCORES          # sequences per core
T = BLOC * L                # tokens per core
DT = D // 128               # 4 partition tiles of the model dim
QT = L // 128               # 4 q-tiles per sequence
TH = T // 512               # 2 token-halves for N<=512 matmuls

f32 = mybir.dt.float32
f32r = mybir.dt.float32r
bf16 = mybir.dt.bfloat16
AF = mybir.ActivationFunctionType
OP = mybir.AluOpType
AX = mybir.AxisListType

# layer meta: (vsrc, excl_mask, zero_pad, ffn)
LAYERS = [
    ("x", False, False, True),   # enc1 L0
    ("x", False, False, True),   # enc1 L1
    ("x", False, False, False),  # enc2 L0 (self, no FFN)
    ("y", True, True, True),     # enc2 L1 (cross)
    ("x", False, False, False),  # enc2 L2
    ("y", True, True, True),     # enc2 L3 (cross)
]
ENC1_LAYERS = 2

# packed per-partition vector columns (in "pb{l}" [128, 44]):
#  0:4   bqk/sqrt(8)   4:8  bo   8:12 ln1s  12:16 ln1b
# 16:32  b1           32:36 b2  36:40 ln2s  40:44 ln2b
PB_COLS = 44


# ================================================================ builder
def build_nc():
    nc = bacc.Bacc("TRN2", target_bir_lowering=False, debug=False,
                   num_devices=NCORES)

    def din(name, shape, dt):
        return nc.dram_tensor(name, shape, dt, kind="ExternalInput").ap()

    iemb = din("iemb", [D, T], bf16)
    qemb = din("qemb", [D, T], bf16)
    pos = din("pos", [128, QT * 512], bf16)
    gvec = [din(f"gvec{l}", [128, H], f32) for l in range(6)]
    onescol = din("onescol", [128, 1], bf16)
    onesrow = din("onesrow", [1, 128], f32r)
    zfirst = din("zfirst", [1, 512], bf16)   # ones with [0]=0
    onesfull = din("onesfull", [1, 512], bf16)
    wqk = [din(f"wqk{l}", [D, D], bf16) for l in range(6)]
    wv = [din(f"wv{l}", [D, D], bf16) for l in range(6)]
    wo = [din(f"wo{l}", [D, D], bf16) for l in range(6)]
    w1 = [din(f"w1{l}", [D, FF], bf16) if LAYERS[l][3] else None
          for l in range(6)]
    w2 = [din(f"w2{l}", [FF, D], bf16) if LAYERS[l][3] else None
          for l in range(6)]
    bv = [din(f"bv{l}", [1, D], bf16) for l in range(6)]
    pb = [din(f"pb{l}", [128, PB_COLS], f32) for l in range(6)]
    hw1 = din("hw1", [2 * D, FC], bf16)
    hw2 = din("hw2", [FC, 256], bf16)
    hw3 = din("hw3", [256, 1], bf16)
    hpb = din("hpb", [128, 8], f32)  # 0:4 hb1, 4:6 hb2, 6 hb3
    out = nc.dram_tensor("out", [1, T], f32, kind="ExternalOutput").ap()

    with TileContext(nc) as tc:
        _body(nc, tc, locals())
    nc.finalize()
    return nc


def _body(nc, tc, g):
    iemb, qemb, pos, gvec = g["iemb"], g["qemb"], g["pos"], g["gvec"]
    onescol, onesrow, zfirst, onesfull = (g["onescol"], g["onesrow"],
                                          g["zfirst"], g["onesfull"])
    wqk, wv, wo, w1, w2, bv, pb = (g["wqk"], g["wv"], g["wo"], g["w1"],
                                   g["w2"], g["bv"], g["pb"])
    hw1, hw2, hw3, hpb, out = g["hw1"], g["hw2"], g["hw3"], g["hpb"], g["out"]

    import contextlib
    ctx = contextlib.ExitStack()
    with ctx:
        pool = ctx.enter_context(tc.tile_pool(name="sb", bufs=1))
        pw = ctx.enter_context(tc.tile_pool(name="pw", bufs=4))
        pw1 = ctx.enter_context(tc.tile_pool(name="pw1", bufs=4))
        pw2 = ctx.enter_context(tc.tile_pool(name="pw2", bufs=16))
        pstream = ctx.enter_context(tc.tile_pool(name="pstream", bufs=8))
        pqk = ctx.enter_context(tc.tile_pool(name="pqk", bufs=4))
        po = ctx.enter_context(tc.tile_pool(name="po", bufs=4))
        pv = ctx.enter_context(tc.tile_pool(name="pv", bufs=8))
        pchain = ctx.enter_context(tc.tile_pool(name="pchain", bufs=2))
        ppt = ctx.enter_context(tc.tile_pool(name="ppt", bufs=2))
        psml = ctx.enter_context(tc.tile_pool(name="psml", bufs=4))
        pf1 = ctx.enter_context(tc.tile_pool(name="pf1", bufs=3))
        pxc = ctx.enter_context(tc.tile_pool(name="pxc", bufs=2))
        pdram = ctx.enter_context(tc.tile_pool(name="pdram", bufs=8,
                                               space="DRAM"))
        pydram = ctx.enter_context(tc.tile_pool(name="pydram", bufs=4,
                                                space="DRAM"))
        ps_proj = ctx.enter_context(tc.tile_pool(name="ps_proj", bufs=2,
                                                 space="PSUM"))
        ln_counter = [0]

        # ---------------- persistent constants
        pos_sb = pool.tile([128, QT * 512], bf16, tag="pos_sb", name="pos_sb")
        nc.sync.dma_start(pos_sb[:], pos[:])
        oc_sb = pool.tile([128, 1], bf16, tag="oc_sb", name="oc_sb")
        nc.sync.dma_start(oc_sb[:], onescol[:])
        or_sb = pool.tile([1, 128], f32r, tag="or_sb", name="or_sb")
        nc.sync.dma_start(or_sb[:], onesrow[:])
        zf_sb = pool.tile([1, 512], bf16, tag="zf_sb", name="zf_sb")
        nc.sync.dma_start(zf_sb[:], zfirst[:])
        of_sb = pool.tile([1, 512], bf16, tag="of_sb", name="of_sb")
        nc.sync.dma_start(of_sb[:], onesfull[:])
        eps_sb = pool.tile([128, 1], f32, tag="eps_sb", name="eps_sb")
        nc.gpsimd.memset(eps_sb[:], 1e-5)

        # ---------------- helpers
        def load_w(dram, n_tiles, width, mypool, tagp):
            ts = []
            for kc in range(n_tiles):
                t = mypool.tile([128, width], bf16, tag=f"{tagp}", name=f"{tagp}")
                nc.sync.dma_start(t[:], dram[kc * 128:(kc + 1) * 128, :])
                ts.append(t)
            return ts

        def proj_B(wt, src, dst_tag, dst_pool, bias=None, act=AF.Identity,
                   out_dt=bf16, residual=None, res_c=None, n_out=DT):
            """dst[do,t] = act(sum_di W[di,do] src[di,t] + bias).
            If residual is given: dst = (psum + res_c) + residual (STT)."""
            dst = []
            for dt_i in range(n_out):
                d = dst_pool.tile([128, T], out_dt, tag=dst_tag, name=dst_tag)
                dst.append(d)
                for th in range(TH):
                    ps = ps_proj.tile([128, 512], f32, tag="ps_proj", name="ps_proj")
                    for kc in range(len(wt)):
                        nc.tensor.matmul(
                            ps[:], wt[kc][:, dt_i * 128:(dt_i + 1) * 128],
                            src[kc][:, th * 512:(th + 1) * 512],
                            start=(kc == 0), stop=(kc == len(wt) - 1))
                    sl = d[:, th * 512:(th + 1) * 512]
                    if residual is not None:
                        nc.vector.scalar_tensor_tensor(
                            sl, ps[:], res_c[:, dt_i:dt_i + 1],
                            residual[dt_i][:, th * 512:(th + 1) * 512],
                            OP.add, OP.add)
                    else:
                        b = 0.0 if bias is None else bias[:, dt_i:dt_i + 1]
                        nc.scalar.activation(sl, ps[:], act, bias=b)
            return dst

        def layer_norm(src, pbt, s_col, b_col):
            """feature-major layernorm over partitions (4 tiles)."""
            lnid = ln_counter[0]
            ln_counter[0] += 1
            with tc.tile_pool(name=f"ps_ln{lnid}", bufs=2,
                              space="PSUM") as ps_ln:
                s1sb = psml.tile([1, T], f32, tag="s1sb", name="s1sb", bufs=2)
                s2sb = psml.tile([1, T], f32, tag="s2sb", name="s2sb", bufs=2)
                for th in range(TH):
                    sl = slice(th * 512, (th + 1) * 512)
                    s1 = ps_ln.tile([1, 512], f32, tag="ps_stat", name="ps_stat")
                    for kc in range(DT):
                        nc.tensor.matmul(s1[0:1, :], oc_sb[:],
                                         src[kc][:, sl],
                                         start=(kc == 0), stop=(kc == DT - 1))
                    nc.scalar.copy(s1sb[0:1, sl], s1[0:1, :])
                    s2 = ps_ln.tile([1, 512], f32, tag="ps_stat", name="ps_stat")
                    for kc in range(DT):
                        rsq = pxc.tile([128, 512], bf16, tag="rsq", name="rsq")
                        nc.scalar.square(rsq[:], src[kc][:, sl])
                        nc.tensor.matmul(s2[0:1, :], oc_sb[:], rsq[:],
                                         start=(kc == 0), stop=(kc == DT - 1))
                    nc.scalar.copy(s2sb[0:1, sl], s2[0:1, :])
                tmu = psml.tile([1, T], f32r, tag="tmu", name="tmu", bufs=2)
                nc.vector.tensor_scalar_mul(tmu[:], s1sb[:], 1.0 / D)
                tms = psml.tile([1, T], f32, tag="lnt", name="tms", bufs=3)
                nc.vector.tensor_mul(tms[:], tmu[:], tmu[:])
                tvar = psml.tile([1, T], f32, tag="lnt", name="tvar", bufs=3)
                nc.vector.scalar_tensor_tensor(tvar[:], s2sb[:], 1.0 / D,
                                               tms[:], OP.mult, OP.subtract)
                tstd = psml.tile([1, T], f32, tag="lnt", name="tstd", bufs=3)
                nc.scalar.activation(tstd[:], tvar[:], AF.Sqrt,
                                     bias=eps_sb[0:1, 0:1])
                trs = psml.tile([1, T], f32, tag="lnt", name="trs", bufs=3)
                nc.vector.reciprocal(trs[:], tstd[:])
                trsr = psml.tile([1, T], f32r, tag="trsr", name="trsr", bufs=2)
                nc.scalar.copy(trsr[:], trs[:])

                dst = []
                for dt_i in range(DT):
                    dst.append(pstream.tile([128, T], bf16, tag="stream",
                                            name="stream"))
                for th in range(TH):
                    sl = slice(th * 512, (th + 1) * 512)
                    mub = ps_ln.tile([128, 512], f32, tag="ps_bc", name="ps_bc")
                    nc.tensor.matmul(mub[:], or_sb[:], tmu[0:1, sl],
                                     start=True, stop=True)
                    rsb = ps_ln.tile([128, 512], f32, tag="ps_bc", name="ps_bc")
                    nc.tensor.matmul(rsb[:], or_sb[:], trsr[0:1, sl],
                                     start=True, stop=True)
                    for dt_i in range(DT):
                        xc = pxc.tile([128, 512], bf16, tag="xc", name="xc")
                        nc.vector.tensor_sub(xc[:], src[dt_i][:, sl], mub[:])
                        y0 = pxc.tile([128, 512], bf16, tag="y0", name="y0")
                        nc.vector.scalar_tensor_tensor(
                            y0[:], xc[:],
                            pbt[:, s_col + dt_i:s_col + dt_i + 1],
                            rsb[:], OP.mult, OP.mult)
                        nc.vector.tensor_scalar_add(
                            dst[dt_i][:, sl], y0[:],
                            pbt[:, b_col + dt_i:b_col + dt_i + 1])
            return dst

        # ---------------- input stream
        x = []
        for dt_i in range(DT):
            t = pstream.tile([128, T], bf16, tag="stream", name="stream")
            nc.sync.dma_start(t[:], iemb[dt_i * 128:(dt_i + 1) * 128, :])
            x.append(t)
        y_dram = None

        for li, (vsrc, excl, zpad, ffn) in enumerate(LAYERS):
            if li == ENC1_LAYERS:
                # store y = enc1 output to DRAM, switch stream to q_emb
                y_dram = []
                for dt_i in range(DT):
                    yd = pydram.tile([128, T], bf16, tag="ydram", name="ydram")
                    nc.sync.dma_start(yd[:], x[dt_i][:])
                    y_dram.append(yd)
                x = []
                for dt_i in range(DT):
                    t = pstream.tile([128, T], bf16, tag="stream", name="stream")
                    nc.sync.dma_start(t[:],
                                      qemb[dt_i * 128:(dt_i + 1) * 128, :])
                    x.append(t)

            pbt = pool.tile([128, PB_COLS], f32, tag=f"pb{li}", name=f"pb{li}")
            nc.sync.dma_start(pbt[:], pb[li][:])
            bv_sb = pool.tile([1, D], bf16, tag=f"bv{li}", name=f"bv{li}")
            nc.sync.dma_start(bv_sb[:], bv[li][:])
            gv_sb = pool.tile([128, H], f32, tag=f"gv{li}", name=f"gv{li}")
            nc.sync.dma_start(gv_sb[:], gvec[li][:])

            wqk_t = load_w(wqk[li], DT, 512, pw, "wqk")
            wv_t = load_w(wv[li], DT, 512, pw, "wv")
            wo_t = load_w(wo[li], DT, 512, pw, "wo")

            # q==k projection (shared, pre-scaled by 1/sqrt(8))
            qk = proj_B(wqk_t, x, "qk", pqk, bias=pbt[:, 0:4])

            # V token-major (A-mode): v_sb[tt] = [128 tok, 512 dv] bf16
            if vsrc == "y":
                vsrc_t = []
                for dt_i in range(DT):
                    t = pxc.tile([128, T], bf16, tag="ysrc", name="ysrc", bufs=4)
                    nc.sync.dma_start(t[:], y_dram[dt_i][:])
                    vsrc_t.append(t)
            else:
                vsrc_t = x
            v_sb = []
            for tt in range(T // 128):
                ps = ps_proj.tile([128, 512], f32, tag="ps_proj", name="ps_proj")
                for kc in range(DT):
                    nc.tensor.matmul(ps[:],
                                     vsrc_t[kc][:, tt * 128:(tt + 1) * 128],
                                     wv_t[kc][:],
                                     start=(kc == 0), stop=(kc == DT - 1))
                vt = pv.tile([128, 512], bf16, tag="v", name="v")
                nc.scalar.copy(vt[:], ps[:])
                v_sb.append(vt)

            # attention
            att_ctx = contextlib.ExitStack()
            ps_s = att_ctx.enter_context(tc.tile_pool(
                name=f"ps_s{li}", bufs=2, space="PSUM"))
            ps_av = att_ctx.enter_context(tc.tile_pool(
                name=f"ps_av{li}", bufs=2, space="PSUM"))
            o_t = [po.tile([128, T], bf16, tag="o", name="o") for _ in range(DT)]
            for b in range(BLOC):
                for h in range(H):
                    ho = (h % 2) * 64
                    qk_t = qk[h // 2]
                    pds = []
                    for qi in range(QT):
                        W = 128 * (qi + 1)
                        sps = ps_s.tile([128, 512], f32, tag="ps_s", name="ps_s")
                        nc.tensor.matmul(
                            sps[:, :W],
                            qk_t[ho:ho + 64,
                                 b * 512 + qi * 128:b * 512 + (qi + 1) * 128],
                            qk_t[ho:ho + 64, b * 512:b * 512 + W],
                            start=True, stop=True, tile_position=(ho, 0))
                        S = pchain.tile([128, 512], bf16, tag="S", name="S")
                        if qi % 2 == 0:
                            nc.scalar.copy(S[:, :W], sps[:, :W])
                        else:
                            nc.vector.tensor_copy(S[:, :W], sps[:, :W])
                        # causal mask on the diagonal block
                        dsl = slice(qi * 128, (qi + 1) * 128)
                        nc.gpsimd.affine_select(
                            out=S[:, dsl], in_=S[:, dsl],
                            compare_op=OP.is_ge, fill=-1e30,
                            base=(-1 if excl else 0),
                            pattern=[[-1, 128]], channel_multiplier=1)
                        e = pchain.tile([128, 512], bf16, tag="e", name="e")
                        den = psml.tile([128, 1], f32, tag="den", name="den")
                        nc.scalar.activation(e[:, :W], S[:, :W], AF.Exp,
                                             accum_out=den[:])
                        rden = psml.tile([128, 1], f32, tag="rden", name="rden")
                        nc.vector.reciprocal(rden[:], den[:])
                        nrden = psml.tile([128, 1], f32, tag="nrden", name="nrden")
                        nc.vector.tensor_scalar_mul(nrden[:], rden[:], -1.0)
                        cum = pchain.tile([128, 512], bf16, tag="cum", name="cum")
                        nc.vector.tensor_tensor_scan(
                            cum[:, :W], e[:, :W], e[:, :W], 0.0,
                            OP.add, OP.bypass)
                        t1 = pchain.tile([128, 512], bf16, tag="t1", name="t1")
                        nc.vector.tensor_scalar(t1[:, :W], cum[:, :W],
                                                den[:], nrden[:],
                                                OP.subtract, OP.mult)
                        d2 = pchain.tile([128, 512], bf16, tag="d2", name="d2")
                        nc.vector.scalar_tensor_tensor(
                            d2[:, :W], t1[:, :W], 0.0,
                            pos_sb[:, qi * 512:qi * 512 + W],
                            OP.max, OP.mult)
                        nc.scalar.sqrt(d2[:, :W], d2[:, :W])
                        te = pchain.tile([128, 512], bf16, tag="te", name="te")
                        nc.scalar.activation(te[:, :W], d2[:, :W], AF.Exp,
                                             scale=gv_sb[:, h:h + 1])
                        s2 = pchain.tile([128, 512], bf16, tag="s2", name="s2")
                        nc.vector.scalar_tensor_tensor(
                            s2[:, :W], te[:, :W], 1e-5, S[:, :W],
                            OP.max, OP.mult)
                        e2 = pchain.tile([128, 512], bf16, tag="e2", name="e2")
                        den2 = psml.tile([128, 1], f32, tag="den2", name="den2")
                        nc.scalar.activation(e2[:, :W], s2[:, :W], AF.Exp,
                                             accum_out=den2[:])
                        rden2 = psml.tile([128, 1], f32, tag="rden2", name="rden2")
                        nc.vector.reciprocal(rden2[:], den2[:])
                        P = pchain.tile([128, 512], bf16, tag="P", name="P")
                        nc.vector.tensor_scalar_mul(P[:, :W], e2[:, :W],
                                                    rden2[:])
                        if zpad and qi == 0:
                            nc.gpsimd.memset(P[0:1, :W], 0.0)
                        pd = pdram.tile([128, 512], bf16, tag="pd", name="pd")
                        nc.sync.dma_start(pd[:, :W], P[:, :W])
                        pds.append(pd)
                    # transpose P blocks (DRAM -> SBUF xbar transpose)
                    pt = []
                    for kj in range(QT):
                        ptt = ppt.tile([128, 512], bf16, tag=f"pt{kj}", name=f"pt{kj}")
                        for qi in range(kj, QT):
                            nc.sync.dma_start_transpose(
                                ptt[:, qi * 128:(qi + 1) * 128],
                                pds[qi][:, kj * 128:(kj + 1) * 128])
                        pt.append(ptt)
                    # PV matmul (+ bv via K=1 trick)
                    if h % 2 == 0:
                        avps = ps_av.tile([128, 512], f32, tag="ps_av", name="ps_av")
                    nc.tensor.matmul(
                        avps[ho:ho + 64, :],
                        bv_sb[0:1, h * 64:(h + 1) * 64],
                        (zf_sb if zpad else of_sb)[0:1, :],
                        start=True, stop=False, tile_position=(0, ho))
                    for qi in range(QT):
                        for kj in range(qi + 1):
                            nc.tensor.matmul(
                                avps[ho:ho + 64,
                                     qi * 128:(qi + 1) * 128],
                                v_sb[b * 4 + kj][:, h * 64:(h + 1) * 64],
                                pt[kj][:, qi * 128:(qi + 1) * 128],
                                start=False, stop=(kj == qi),
                                tile_position=(0, ho))
                    if h % 2 == 1:
                        nc.scalar.copy(
                            o_t[h // 2][:, b * 512:(b + 1) * 512], avps[:])

            att_ctx.close()
            # output projection + residual
            r = proj_B(wo_t, o_t, "stream", pstream, residual=x,
                       res_c=pbt[:, 4:8])
            y1 = layer_norm(r, pbt, 8, 12)

            if ffn:
                f2_ctx = contextlib.ExitStack()
                ps_f2 = f2_ctx.enter_context(tc.tile_pool(
                    name=f"ps_f2{li}", bufs=4, space="PSUM"))
                w1_t = load_w(w1[li], DT, FF, pw1, "w1")
                w2_t = load_w(w2[li], FF // 128, 512, pw2, "w2")
                r2 = []
                for dt_i in range(DT):
                    r2.append(pstream.tile([128, T], bf16, tag="stream", name="stream"))
                for th in range(TH):
                    sl = slice(th * 512, (th + 1) * 512)
                    f2ps = [ps_f2.tile([128, 512], f32, tag="ps_f2", name="ps_f2")
                            for _ in range(DT)]
                    for fo in range(FF // 128):
                        ps1 = ps_proj.tile([128, 512], f32, tag="ps_proj", name="ps_proj")
                        for kc in range(DT):
                            nc.tensor.matmul(
                                ps1[:],
                                w1_t[kc][:, fo * 128:(fo + 1) * 128],
                                y1[kc][:, sl],
                                start=(kc == 0), stop=(kc == DT - 1))
                        f1c = pf1.tile([128, 512], bf16, tag="f1", name="f1")
                        nc.scalar.activation(f1c[:], ps1[:], AF.Relu,
                                             bias=pbt[:, 16 + fo:17 + fo])
                        for dt_i in range(DT):
                            nc.tensor.matmul(
                                f2ps[dt_i][:],
                                w2_t[fo][:, dt_i * 128:(dt_i + 1) * 128],
                                f1c[:],
                                start=(fo == 0), stop=(fo == FF // 128 - 1))
                    for dt_i in range(DT):
                        nc.vector.scalar_tensor_tensor(
                            r2[dt_i][:, sl], f2ps[dt_i][:],
                            pbt[:, 32 + dt_i:33 + dt_i], y1[dt_i][:, sl],
                            OP.add, OP.add)
                f2_ctx.close()
                x = layer_norm(r2, pbt, 36, 40)
            else:
                x = y1

        # ---------------- prediction head
        hw1_t = load_w(hw1, 2 * DT, 512, pw2, "w2")
        hw2_t = load_w(hw2, DT, 256, pw, "wqk")
        hw3_t = load_w(hw3, 2, 1, pw, "wv")
        hpb_sb = pool.tile([128, 8], f32, tag="hpb_sb", name="hpb_sb")
        nc.sync.dma_start(hpb_sb[:], hpb[:])
        qe2 = []
        for dt_i in range(DT):
            t = pxc.tile([128, T], bf16, tag="ysrc", name="ysrc", bufs=4)
            nc.sync.dma_start(t[:], qemb[dt_i * 128:(dt_i + 1) * 128, :])
            qe2.append(t)
        src8 = x + qe2
        h1 = proj_B(hw1_t, src8, "o", po, bias=hpb_sb[:, 0:4], act=AF.Relu)
        h2 = []
        for dt_i in range(2):
            d = pf1.tile([128, T], bf16, tag="h2", name="h2")
            h2.append(d)
            for th in range(TH):
                ps = ps_proj.tile([128, 512], f32, tag="ps_proj", name="ps_proj")
                for kc in range(DT):
                    nc.tensor.matmul(ps[:],
                                     hw2_t[kc][:, dt_i * 128:(dt_i + 1) * 128],
                                     h1[kc][:, th * 512:(th + 1) * 512],
                                     start=(kc == 0), stop=(kc == DT - 1))
                nc.scalar.activation(d[:, th * 512:(th + 1) * 512], ps[:],
                                     AF.Relu, bias=hpb_sb[:, 4 + dt_i:5 + dt_i])
        with tc.tile_pool(name="ps_head", bufs=1, space="PSUM") as ps_head:
            lps = ps_head.tile([1, T], f32, tag="lps", name="lps")
            for th in range(TH):
                sl = slice(th * 512, (th + 1) * 512)
                for kc in range(2):
                    nc.tensor.matmul(lps[0:1, sl], hw3_t[kc][:],
                                     h2[kc][:, sl],
                                     start=(kc == 0), stop=(kc == 1))
            sig = psml.tile([1, T], f32, tag="sig", name="sig", bufs=1)
            nc.scalar.activation(sig[:], lps[:], AF.Sigmoid,
                                 bias=hpb_sb[0:1, 6:7])
            nc.sync.dma_start(out[:], sig[:])


# ================================================================ host side
def _host_prep(params, concept_seq, question_seq, correct_seq):
    """Rasch embeddings + weight layout prep (all numpy)."""
    p = {k: np.asarray(v) for k, v in params.items()
         if not isinstance(v, (list, dict))}
    Ec, Ecv, Eiv, Ei, Eqd = (np.asarray(params["Ec"]),
                             np.asarray(params["Ecv"]),
                             np.asarray(params["Eiv"]),
                             np.asarray(params["Ei"]),
                             np.asarray(params["Eqd"]))
    cs = np.asarray(concept_seq)
    qs = np.asarray(question_seq)
    rs = np.asarray(correct_seq)

    c_emb = Ec[cs]                       # [B,L,D]
    cv_emb = Ecv[cs]
    i_emb = Ei[rs] + c_emb
    qd = Eqd[qs]                         # [B,L,1]
    q_emb = c_emb + qd * cv_emb
    iv_emb = Eiv[cs + C * rs]
    i_emb = i_emb + qd * (iv_emb + cv_emb)

    layers = list(params["enc1"]) + list(params["enc2"])
    f32n = np.float32
    bfn = ml_dtypes.bfloat16

    base = {}
    base["pos"] = np.zeros((128, QT * 512), np.float32)
    for qi in range(QT):
        i = (128 * qi + np.arange(128))[:, None]
        j = np.arange(512)[None, :]
        base["pos"][:, qi * 512:(qi + 1) * 512] = np.abs(i - j)
    base["pos"] = base["pos"].astype(bfn)
    base["onescol"] = np.ones((128, 1), f32n)
    base["onesrow"] = np.ones((1, 128), f32n)
    zf = np.ones((1, 512), f32n)
    zf[0, 0] = 0.0
    base["zfirst"] = zf.astype(bfn)
    base["onesfull"] = np.ones((1, 512), bfn)

    base["onescol"] = np.ones((128, 1), bfn)
    sc = 1.0 / np.sqrt(np.sqrt(np.float32(DH)))  # fold 1/sqrt(DH) into W,b
    for li, lp in enumerate(layers):
        base[f"wqk{li}"] = (np.asarray(lp["Wqk"]) * sc * sc).astype(bfn)
        base[f"wv{li}"] = np.asarray(lp["Wv"]).astype(bfn)
        base[f"wo{li}"] = np.asarray(lp["Wo"]).astype(bfn)
        if LAYERS[li][3]:
            base[f"w1{li}"] = np.asarray(lp["W1"]).astype(bfn)
            base[f"w2{li}"] = np.asarray(lp["W2"]).astype(bfn)
        base[f"bv{li}"] = np.asarray(lp["bv"]).reshape(1, D).astype(bfn)
        pbt = np.zeros((128, PB_COLS), f32n)
        pbt[:, 0:4] = (np.asarray(lp["bqk"]) * sc * sc).reshape(4, 128).T
        pbt[:, 4:8] = np.asarray(lp["bo"]).reshape(4, 128).T
        pbt[:, 8:12] = np.asarray(lp["ln1s"]).reshape(4, 128).T
        pbt[:, 12:16] = np.asarray(lp["ln1b"]).reshape(4, 128).T
        if LAYERS[li][3]:
            pbt[:, 16:32] = np.asarray(lp["b1"]).reshape(16, 128).T
            pbt[:, 32:36] = np.asarray(lp["b2"]).reshape(4, 128).T
            pbt[:, 36:40] = np.asarray(lp["ln2s"]).reshape(4, 128).T
            pbt[:, 40:44] = np.asarray(lp["ln2b"]).reshape(4, 128).T
        base[f"pb{li}"] = pbt
        ga = np.asarray(lp["gamma"], np.float64)
        g = -(np.logaddexp(0.0, ga))
        base[f"gvec{li}"] = np.broadcast_to(
            g.astype(f32n)[None, :], (128, H)).copy()

    hp = params["head"]
    base["hw1"] = np.asarray(hp["W1"]).astype(bfn)
    base["hw2"] = np.asarray(hp["W2"]).astype(bfn)
    base["hw3"] = np.asarray(hp["W3"]).astype(bfn)
    hpb = np.zeros((128, 8), f32n)
    hpb[:, 0:4] = np.asarray(hp["b1"]).reshape(4, 128).T
    hpb[:, 4:6] = np.asarray(hp["b2"]).reshape(2, 128).T
    hpb[0, 6] = float(np.asarray(hp["b3"])[0])
    base["hpb"] = hpb

    in_maps = []
    for c in range(NCORES):
        m = dict(base)
        sl = slice(c * BLOC, (c + 1) * BLOC)
        m["iemb"] = np.ascontiguousarray(
            i_emb[sl].reshape(T, D).T).astype(bfn)
        m["qemb"] = np.ascontiguousarray(
            q_emb[sl].reshape(T, D).T).astype(bfn)
        in_maps.append(m)
    return in_maps


_NC_CACHE = {}


def kernel(params, concept_seq, question_seq, correct_seq):
    if "nc" not in _NC_CACHE:
        _NC_CACHE["nc"] = build_nc()
    nc = _NC_CACHE["nc"]
    in_maps = _host_prep(params, concept_seq, question_seq, correct_seq)
    res = run_bass_kernel_spmd(nc, in_maps, core_ids=list(range(NCORES)))
    outs = [res.results[c]["out"].reshape(BLOC, L) for c in range(NCORES)]
    return np.concatenate(outs, axis=0).astype(np.float32)
